# revision 15
# baseline (speedup 1.0000x reference)
"""Trainium2 Bass kernel: 1-layer transformer block w/ ALiBi bidirectional attention.

Sharding: data-parallel over batch (B=8) across 8 NeuronCores; zero collectives.

v2 (bf16): all matmuls run in bf16 (1 cyc/row on PE vs 4 for fp32, and
LDWEIGHTS gets fast-weight-load). Residual stream / LN stats / softmax
normalization stay fp32. Other changes vs v1:
  - x is transposed + cast to bf16 on HOST; no on-device xT transposes.
  - q/k projections emit per-head [64, S] psum chunks directly into the
    augmented [65, S] qTa/kTa tiles -> no SBUF->SBUF head-split DMAs.
  - probs@V computed s-major: out[s, 65] = expT_chunk.T @ v_aug, so the
    softmax denominator lands in column 64 and normalization is a
    per-partition scalar multiply; no per-head transposes.
  - ALiBi: per-s term rides the augmented q row (bf16 rounding of it is a
    per-s additive exponent error that cancels exactly in softmax);
    per-t term is the fp32 per-partition ACT bias of the fused exp.
  - LN scale/bias folded into following weight matrices host-side.
"""

import sys

import ml_dtypes
import numpy as np

sys.path.insert(0, "/opt/trn_rl_repo")

import concourse.bass as bass  # noqa: E402
from concourse import bacc  # noqa: E402
import concourse.tile as tile  # noqa: E402
from concourse import mybir  # noqa: E402
from concourse.bass_utils import run_bass_kernel_spmd  # noqa: E402

F32 = mybir.dt.float32
BF = mybir.dt.bfloat16
AF = mybir.ActivationFunctionType
OP = mybir.AluOpType

P = 128
B = 8
S = 1024
D = 512
H = 8
HD = 64
FFN = 4 * D
SM = S // P  # 8 sequence chunks
DK = D // P  # 4 feature chunks
FK = FFN // P  # 16 ffn chunks
EPS = 1e-5
N_CORES = 8

BF_NP = ml_dtypes.bfloat16


def _slopes():
    half = H // 2
    base = 24.0 ** (1.0 / half)
    return (1.0 / base ** np.arange(1, half + 1)).astype(np.float64)


def _fwd(h):
    return h < H // 2


# per (head, j) score-tile geometry for the transposed scores [t=j*128+p, s]
def _s_range(h, j):
    if _fwd(h):  # keep t <= s : s-chunks j..7
        return j * P, S - j * P
    else:  # keep t >= s : s-chunks 0..j
        return 0, (j + 1) * P


def _eoff(h, j):
    off = 0
    for jj in range(j):
        off += _s_range(h, jj)[1]
    return off


def _ewidth(h):
    return _eoff(h, SM - 1) + _s_range(h, SM - 1)[1]  # = 4608


def build_nc(gelu_mode="gelu"):
    nc = bacc.Bacc("TRN2", target_bir_lowering=False, debug=False)

    def din(name, shape, dt=F32):
        return nc.dram_tensor(name, list(shape), dt, kind="ExternalInput").ap()

    d = {}
    d["xT"] = din("xT", (D, S), BF)
    d["w_in"] = din("w_in", (D, D), BF)
    d["b_in"] = din("b_in", (D,))
    d["wq"] = din("wq", (D, D), BF)
    d["wk"] = din("wk", (D, D), BF)
    d["wv"] = din("wv", (D, D), BF)
    d["wo"] = din("wo", (D, D), BF)
    d["bo"] = din("bo", (D,))
    d["w1"] = din("w1", (D, FFN), BF)
    d["w2"] = din("w2", (FFN, D), BF)
    d["b2"] = din("b2", (D,))
    d["w_out"] = din("w_out", (D, D), BF)
    d["b_out"] = din("b_out", (D,))
    d["bqc"] = din("bqc", (HD, H))
    d["b1c"] = din("b1c", (P, FK))
    d["bv"] = din("bv", (D,))
    d["qrow"] = din("qrow", (H, S), BF)
    d["tb"] = din("tb", (P, H * SM))
    d["maskf"] = din("maskf", (P, P), BF)
    d["maskb"] = din("maskb", (P, P), BF)
    d["ident"] = din("ident", (P, P), BF)
    d["out"] = nc.dram_tensor("out", [S, D], F32, kind="ExternalOutput").ap()

    with tile.TileContext(nc) as tc:
        _emit(nc, tc, d, gelu_mode)
    nc.compile()
    return nc


def _emit(nc, tc, d, gelu_mode):
    pool = tc.alloc_tile_pool

    pc = pool(name="consts", bufs=1)
    pw = pool(name="weights", bufs=1)  # all weights resident, bf16
    ph = pool(name="resid", bufs=2)  # tag "h": h1, h2, h3 rotate (fp32)
    phT = pool(name="transposed", bufs=2)  # tag "hT": hn1T,attnT2,hn2T,hn3T
    psm = pool(name="smalls", bufs=4)
    phn = pool(name="hn_nat", bufs=2)
    pg = pool(name="gelu", bufs=3)
    posb = pool(name="outsb", bufs=3)
    pattn = pool(name="attn_nat", bufs=1)
    pva = pool(name="vaug", bufs=1)
    pqk = pool(name="qkheads", bufs=2)
    pexp = pool(name="expT", bufs=2)

    ps_mm = pool(name="ps_mm", bufs=2, space="PSUM")
    # 4-deep rotation: score matmuls run ahead of ACT exp; doubles as the
    # 4 live FFN2 accumulators
    ps_acc = pool(name="ps_acc", bufs=4, space="PSUM")
    ps_tr = pool(name="ps_tr", bufs=2, space="PSUM")

    # ---- weights (bf16), staged early; all fit resident ----
    def wload(name, shape, view):
        t = pw.tile(shape, BF, tag=name)
        nc.sync.dma_start(out=t, in_=view)
        return t

    win_sb = wload("w_in", [P, DK, D], d["w_in"].rearrange("(c p) n -> p c n", p=P))
    # x arrives in s-chunks so h1(m=0) can start after 1/8 of the load
    xT_sb = pw.tile([P, DK, S], BF, tag="xT")
    xT_view = d["xT"].rearrange("(c p) n -> p c n", p=P)
    for m in range(SM):
        nc.sync.dma_start(
            out=xT_sb[:, :, m * P : (m + 1) * P],
            in_=xT_view[:, :, m * P : (m + 1) * P],
        )
    wq_sb = wload("wq", [P, DK, D], d["wq"].rearrange("(c p) n -> p c n", p=P))
    wk_sb = wload("wk", [P, DK, D], d["wk"].rearrange("(c p) n -> p c n", p=P))
    wv_sb = wload("wv", [P, DK, D], d["wv"].rearrange("(c p) n -> p c n", p=P))
    wo_sb = wload("wo", [P, DK, D], d["wo"].rearrange("(c p) n -> p c n", p=P))
    w1_sb = wload("w1", [P, DK, FFN], d["w1"].rearrange("(c p) n -> p c n", p=P))
    w2_sb = wload("w2", [P, FK, D], d["w2"].rearrange("(c p) n -> p c n", p=P))
    wout_sb = wload("w_out", [P, DK, D], d["w_out"].rearrange("(c p) n -> p c n", p=P))

    # ---- constants ----
    identB = pc.tile([P, P], BF, tag="ident")
    nc.sync.dma_start(out=identB, in_=d["ident"])
    maskf = pc.tile([P, P], BF, tag="maskf")
    nc.sync.dma_start(out=maskf, in_=d["maskf"])
    maskb = pc.tile([P, P], BF, tag="maskb")
    nc.sync.dma_start(out=maskb, in_=d["maskb"])
    tb = pc.tile([P, H * SM], F32, tag="tb")
    nc.sync.dma_start(out=tb, in_=d["tb"])
    bqc = pc.tile([HD, H], F32, tag="bqc")
    nc.sync.dma_start(out=bqc, in_=d["bqc"])
    b1c = pc.tile([P, FK], F32, tag="b1c")
    nc.sync.dma_start(out=b1c, in_=d["b1c"])
    b1cs = pc.tile([P, FK], F32, tag="b1cs")
    nc.any.tensor_scalar(b1cs, b1c, scalar1=1.702, scalar2=None, op0=OP.mult)

    def bcast(name, shape=None):
        t = pc.tile(shape or [P, D], F32, tag=name + "B")
        nc.gpsimd.dma_start(out=t, in_=d[name].partition_broadcast(P))
        return t

    epsc = pc.tile([P, 1], F32, tag="epsc")
    nc.any.memset(epsc, EPS)

    binB = bcast("b_in")
    bvB = bcast("bv", [P, H, HD])
    boB = bcast("bo")
    b2B = bcast("b2")
    boutB = bcast("b_out")

    # ---- h1 = x @ w_in + b_in  (natural fp32, residual base) ----
    h1 = ph.tile([P, SM, D], F32, tag="h")
    for m in range(SM):
        ps = ps_mm.tile([P, D], F32, tag="mm")
        for dk in range(DK):
            nc.tensor.matmul(
                ps,
                xT_sb[:, dk, m * P : (m + 1) * P],
                win_sb[:, dk, :],
                start=(dk == 0),
                stop=(dk == DK - 1),
            )
        nc.vector.tensor_tensor(out=h1[:, m, :], in0=ps, in1=binB, op=OP.add)

    def ln_chunk(src):
        # plain LayerNorm (no scale/bias; those are folded into weights)
        stats = psm.tile([P, 6], F32, tag="st")
        nc.vector.bn_stats(stats, src)
        mv = psm.tile([P, 2], F32, tag="mv")
        nc.vector.bn_aggr(mv, stats)
        sq = psm.tile([P, 1], F32, tag="sq")
        nc.scalar.activation(sq, mv[:, 1:2], AF.Sqrt, bias=epsc)
        rstd = psm.tile([P, 1], F32, tag="rstd")
        nc.vector.reciprocal(rstd, sq)
        negmr = psm.tile([P, 1], F32, tag="negmr")
        nc.vector.tensor_scalar(
            negmr, mv[:, 0:1], scalar1=rstd, scalar2=-1.0, op0=OP.mult, op1=OP.mult
        )
        hn = phn.tile([P, D], BF, tag="hn")
        nc.vector.tensor_scalar(
            hn, src, scalar1=rstd, scalar2=negmr, op0=OP.mult, op1=OP.add
        )
        return hn

    def transpose_row(hT, m, src):
        # transpose the 4 [128,128] blocks of src into one psum tile, then
        # write hT[:, :, m*P:(m+1)*P] with a single strided DVE copy
        t4 = ps_tr.tile([P, DK, P], BF, tag="tr")
        for dk in range(DK):
            nc.tensor.transpose(
                t4[:, dk, :], src[:, dk * P : (dk + 1) * P], identB
            )
        nc.vector.tensor_copy(hT[:, :, m * P : (m + 1) * P], t4)

    def make_hnT(hsrc):
        hT = phT.tile([P, DK, S], BF, tag="hT")
        for m in range(SM):
            hn = ln_chunk(hsrc[:, m, :])
            transpose_row(hT, m, hn)
        return hT

    # hn1T = LN1(h1) transposed [d, s] bf16
    hn1T = make_hnT(h1)

    # ---- v projection -> v_aug [P=t, SM, H, 65] bf16 (ones col for denom) ----
    v_aug = pva.tile([P, SM, H, HD + 1], BF, tag="vaug")
    for t in range(SM):
        psv = ps_mm.tile([P, H, HD], F32, tag="mm", name="psv")
        for dk in range(DK):
            nc.tensor.matmul(
                psv,
                hn1T[:, dk, t * P : (t + 1) * P],
                wv_sb[:, dk, :],
                start=(dk == 0),
                stop=(dk == DK - 1),
            )
        nc.vector.tensor_tensor(out=v_aug[:, t, :, 0:HD], in0=psv, in1=bvB, op=OP.add)
        nc.gpsimd.memset(v_aug[:, t, :, HD : HD + 1], 1.0)

    # ---- attention, head by head ----
    attn_nat = pattn.tile([P, SM, D], BF, tag="attn")
    for h in range(H):
        # q/k projections emitted per-head: psum [64, 512] chunks
        qTa = pqk.tile([HD + 1, S], BF, tag="qTa", name=f"qTa{h}")
        nc.sync.dma_start(out=qTa[HD : HD + 1, :], in_=d["qrow"][h : h + 1, :])
        kTa = pqk.tile([HD + 1, S], BF, tag="kTa", name=f"kTa{h}")
        nc.gpsimd.memset(kTa[HD : HD + 1, :], 1.0)
        for w_sb, dst, is_q in ((wq_sb, qTa, True), (wk_sb, kTa, False)):
            for half in range(2):
                psq = ps_mm.tile([HD, D], F32, tag="mm", name="psq")
                for dk in range(DK):
                    nc.tensor.matmul(
                        psq,
                        w_sb[:, dk, h * HD : (h + 1) * HD],
                        hn1T[:, dk, half * 512 : (half + 1) * 512],
                        start=(dk == 0),
                        stop=(dk == DK - 1),
                    )
                if is_q:
                    nc.vector.tensor_scalar(
                        dst[0:HD, half * 512 : (half + 1) * 512],
                        psq,
                        scalar1=bqc[:, h : h + 1],
                        scalar2=None,
                        op0=OP.add,
                    )
                else:
                    # k bias dropped: it only shifts scores by a per-s
                    # constant, which softmax normalization cancels exactly
                    nc.vector.tensor_copy(
                        dst[0:HD, half * 512 : (half + 1) * 512], psq
                    )

        # scores -> exp, transposed layout [t partitions, s free]
        expT = pexp.tile([P, _ewidth(h)], BF, tag="expT", name=f"expT{h}")
        for j in range(SM):
            s0, w = _s_range(h, j)
            eo = _eoff(h, j)
            off = 0
            while off < w:
                pw_ = min(512, w - off)
                pss = ps_acc.tile([P, pw_], F32, tag="acc", name="pss")
                nc.tensor.matmul(
                    pss,
                    kTa[:, j * P : (j + 1) * P],
                    qTa[:, s0 + off : s0 + off + pw_],
                    start=True,
                    stop=True,
                )
                nc.scalar.activation(
                    expT[:, eo + off : eo + off + pw_],
                    pss,
                    AF.Exp,
                    bias=tb[:, h * SM + j : h * SM + j + 1],
                    scale=0.125,
                )
                off += pw_
            # mask the diagonal 128x128 block (keep t<=s fwd / t>=s bwd)
            dg = eo if _fwd(h) else eo + j * P
            msk = maskf if _fwd(h) else maskb
            nc.gpsimd.tensor_tensor(
                out=expT[:, dg : dg + P],
                in0=expT[:, dg : dg + P],
                in1=msk,
                op=OP.mult,
            )
        # probs @ V, s-major: out[s, 65]; col 64 = softmax denominator
        for m in range(SM):
            js = list(range(0, m + 1)) if _fwd(h) else list(range(m, SM))
            pv = ps_mm.tile([P, HD + 1], F32, tag="mm", name="pvps")
            for i, j in enumerate(js):
                s0, _w = _s_range(h, j)
                col = _eoff(h, j) + (m * P - s0)
                nc.tensor.matmul(
                    pv,
                    expT[:, col : col + P],
                    v_aug[:, j, h, :],
                    start=(i == 0),
                    stop=(i == len(js) - 1),
                )
            rinv = psm.tile([P, 1], F32, tag="rinv")
            nc.vector.reciprocal(rinv, pv[:, HD : HD + 1])
            nc.vector.tensor_scalar(
                attn_nat[:, m, h * HD : (h + 1) * HD],
                pv[:, 0:HD],
                scalar1=rinv,
                scalar2=None,
                op0=OP.mult,
            )

    # attn transposed for the output projection
    attnT2 = phT.tile([P, DK, S], BF, tag="hT")
    for m in range(SM):
        transpose_row(attnT2, m, attn_nat[:, m, :])

    # h2 = h1 + attn @ wo + bo
    h2 = ph.tile([P, SM, D], F32, tag="h")
    for m in range(SM):
        ps = ps_mm.tile([P, D], F32, tag="mm", name="pswo")
        for dk in range(DK):
            nc.tensor.matmul(
                ps,
                attnT2[:, dk, m * P : (m + 1) * P],
                wo_sb[:, dk, :],
                start=(dk == 0),
                stop=(dk == DK - 1),
            )
        nc.vector.tensor_tensor(out=h2[:, m, :], in0=ps, in1=h1[:, m, :], op=OP.add)
        nc.vector.tensor_tensor(out=h2[:, m, :], in0=h2[:, m, :], in1=boB, op=OP.add)

    # hn2T = LN2(h2) transposed bf16
    hn2T = make_hnT(h2)

    # ---- FFN: h3 = h2 + gelu(hn2 @ w1 + b1) @ w2 + b2 ----
    h3 = ph.tile([P, SM, D], F32, tag="h")
    for half in range(2):
        accs = []
        for mm in range(4):
            accs.append(ps_acc.tile([P, D], F32, tag="acc", name=f"ff2ps{mm}"))
        for kc in range(FK):
            ps1 = ps_mm.tile([P, 512], F32, tag="mm", name="ff1ps")
            for dk in range(DK):
                nc.tensor.matmul(
                    ps1,
                    w1_sb[:, dk, kc * P : (kc + 1) * P],
                    hn2T[:, dk, half * 512 : (half + 1) * 512],
                    start=(dk == 0),
                    stop=(dk == DK - 1),
                )
            gt = pg.tile([P, 512], BF, tag="gt")
            if gelu_mode == "gelu":
                nc.scalar.activation(gt, ps1, AF.Gelu, bias=b1c[:, kc : kc + 1])
            else:  # CoreSim lacks Gelu: x*sigmoid(1.702x) stand-in
                sg = pg.tile([P, 512], F32, tag="sg")
                nc.scalar.activation(
                    sg, ps1, AF.Sigmoid, bias=b1cs[:, kc : kc + 1], scale=1.702
                )
                xb = pg.tile([P, 512], F32, tag="xb")
                nc.any.tensor_scalar(
                    xb, ps1, scalar1=b1c[:, kc : kc + 1], scalar2=None, op0=OP.add
                )
                nc.any.tensor_tensor(out=gt, in0=sg, in1=xb, op=OP.mult)
            for mm in range(4):
                nc.tensor.matmul(
                    accs[mm],
                    gt[:, mm * P : (mm + 1) * P],
                    w2_sb[:, kc, :],
                    start=(kc == 0),
                    stop=(kc == FK - 1),
                )
        for mm in range(4):
            m = half * 4 + mm
            nc.vector.tensor_tensor(
                out=h3[:, m, :], in0=accs[mm], in1=h2[:, m, :], op=OP.add
            )
            nc.vector.tensor_tensor(
                out=h3[:, m, :], in0=h3[:, m, :], in1=b2B, op=OP.add
            )

    # ---- final LN + output projection ----
    hn3T = make_hnT(h3)

    out_view = d["out"].rearrange("(c p) n -> p c n", p=P)
    for m in range(SM):
        ps = ps_mm.tile([P, D], F32, tag="mm", name="psout")
        for dk in range(DK):
            nc.tensor.matmul(
                ps,
                hn3T[:, dk, m * P : (m + 1) * P],
                wout_sb[:, dk, :],
                start=(dk == 0),
                stop=(dk == DK - 1),
            )
        osb = posb.tile([P, D], F32, tag="osb")
        nc.vector.tensor_tensor(out=osb, in0=ps, in1=boutB, op=OP.add)
        nc.sync.dma_start(out=out_view[:, m, :], in_=osb)

    for p_ in (ps_tr, ps_acc, ps_mm, pexp, pqk, pva, pattn, posb, pg, phn,
               psm, phT, ph, pw, pc):
        p_.release()


def host_prep(inputs):
    """Fold LN affine params into weights; build ALiBi helper tensors."""
    f = lambda k: np.asarray(inputs[k], dtype=np.float64)
    ln1_s, ln1_b = f("ln1_s"), f("ln1_b")
    ln2_s, ln2_b = f("ln2_s"), f("ln2_b")
    lnf_s, lnf_b = f("lnf_s"), f("lnf_b")
    wq, bq = f("wq"), f("bq")
    wk = f("wk")
    wv, bv = f("wv"), f("bv")
    w1, b1 = f("w1"), f("b1")
    w_out, b_out = f("w_out"), f("b_out")

    wq_f = ln1_s[:, None] * wq
    bq_f = (bq + ln1_b @ wq).astype(np.float32)
    wk_f = ln1_s[:, None] * wk
    wv_f = ln1_s[:, None] * wv
    bv_f = (bv + ln1_b @ wv).astype(np.float32)
    w1_f = ln2_s[:, None] * w1
    b1_f = (b1 + ln2_b @ w1).astype(np.float32)
    wout_f = lnf_s[:, None] * w_out
    bout_f = (b_out + lnf_b @ w_out).astype(np.float32)

    sl = _slopes()
    qrow = np.zeros((H, S), np.float32)
    tb = np.zeros((P, H * SM), np.float32)
    s_idx = np.arange(S, dtype=np.float64)
    p_idx = np.arange(P, dtype=np.float64)
    for h in range(H):
        sgn = -1.0 if h < H // 2 else 1.0  # sign of the per-s row term
        qrow[h] = (sgn * 8.0 * sl[h % 4] * s_idx).astype(np.float32)
        for j in range(SM):
            tb[:, h * SM + j] = (-sgn * sl[h % 4] * (j * P + p_idx)).astype(
                np.float32
            )
    maskf = np.triu(np.ones((P, P), np.float32))  # keep t <= s (p <= c)
    maskb = np.tril(np.ones((P, P), np.float32))  # keep t >= s (p >= c)

    bf = lambda a: np.ascontiguousarray(np.asarray(a, np.float32).astype(BF_NP))
    common = {
        "w_in": bf(inputs["w_in"]),
        "b_in": np.asarray(inputs["b_in"], np.float32),
        "wq": bf(wq_f),
        "wk": bf(wk_f),
        "wv": bf(wv_f),
        "wo": bf(inputs["wo"]),
        "bo": np.asarray(inputs["bo"], np.float32),
        "w1": bf(w1_f),
        "w2": bf(inputs["w2"]),
        "b2": np.asarray(inputs["b2"], np.float32),
        "w_out": bf(wout_f),
        "b_out": bout_f,
        "bqc": np.ascontiguousarray(bq_f.reshape(H, HD).T),
        "b1c": np.ascontiguousarray(b1_f.reshape(FK, P).T),
        "bv": bv_f,
        "qrow": bf(qrow),
        "tb": tb,
        "maskf": bf(maskf),
        "maskb": bf(maskb),
        "ident": bf(np.eye(P, dtype=np.float32)),
    }
    return common


def core_map(common, x, i):
    xT = np.ascontiguousarray(np.asarray(x[i], np.float32).T.astype(BF_NP))
    return dict(common, xT=xT)


_NC_CACHE = {}


def get_nc(gelu_mode="gelu"):
    if gelu_mode not in _NC_CACHE:
        _NC_CACHE[gelu_mode] = build_nc(gelu_mode)
    return _NC_CACHE[gelu_mode]


def run(inputs, trace=False, tmpdir=None):
    common = host_prep(inputs)
    x = np.asarray(inputs["x"], np.float32)
    in_maps = [core_map(common, x, i) for i in range(N_CORES)]
    nc = get_nc()
    res = run_bass_kernel_spmd(
        nc, in_maps, core_ids=list(range(N_CORES)), trace=trace, tmpdir=tmpdir
    )
    out = np.stack([res.results[i]["out"] for i in range(N_CORES)])
    return out.astype(np.float32), res


def kernel(**inputs):
    out, _ = run(inputs, trace=False)
    return out


# revision 17
# speedup vs baseline: 1.1599x; 1.1599x over previous
"""Trainium2 Bass kernel: 1-layer transformer block w/ ALiBi bidirectional attention.

Sharding: data-parallel over batch (B=8) across 8 NeuronCores; zero collectives.

v2 (bf16): all matmuls run in bf16 (1 cyc/row on PE vs 4 for fp32, and
LDWEIGHTS gets fast-weight-load). Residual stream / LN stats / softmax
normalization stay fp32. Other changes vs v1:
  - x is transposed + cast to bf16 on HOST; no on-device xT transposes.
  - q/k projections emit per-head [64, S] psum chunks directly into the
    augmented [65, S] qTa/kTa tiles -> no SBUF->SBUF head-split DMAs.
  - probs@V computed s-major: out[s, 65] = expT_chunk.T @ v_aug, so the
    softmax denominator lands in column 64 and normalization is a
    per-partition scalar multiply; no per-head transposes.
  - ALiBi: per-s term rides the augmented q row (bf16 rounding of it is a
    per-s additive exponent error that cancels exactly in softmax);
    per-t term is the fp32 per-partition ACT bias of the fused exp.
  - LN scale/bias folded into following weight matrices host-side.
"""

import sys

import ml_dtypes
import numpy as np

sys.path.insert(0, "/opt/trn_rl_repo")

import concourse.bass as bass  # noqa: E402
from concourse import bacc  # noqa: E402
import concourse.tile as tile  # noqa: E402
from concourse import mybir  # noqa: E402
from concourse.bass_utils import run_bass_kernel_spmd  # noqa: E402

F32 = mybir.dt.float32
BF = mybir.dt.bfloat16
AF = mybir.ActivationFunctionType
OP = mybir.AluOpType

P = 128
B = 8
S = 1024
D = 512
H = 8
HD = 64
FFN = 4 * D
SM = S // P  # 8 sequence chunks
DK = D // P  # 4 feature chunks
FK = FFN // P  # 16 ffn chunks
EPS = 1e-5
N_CORES = 8

BF_NP = ml_dtypes.bfloat16


def _slopes():
    half = H // 2
    base = 24.0 ** (1.0 / half)
    return (1.0 / base ** np.arange(1, half + 1)).astype(np.float64)


def _fwd(h):
    return h < H // 2


# per (head, j) score-tile geometry for the transposed scores [t=j*128+p, s]
def _s_range(h, j):
    if _fwd(h):  # keep t <= s : s-chunks j..7
        return j * P, S - j * P
    else:  # keep t >= s : s-chunks 0..j
        return 0, (j + 1) * P


def _eoff(h, j):
    off = 0
    for jj in range(j):
        off += _s_range(h, jj)[1]
    return off


def _ewidth(h):
    return _eoff(h, SM - 1) + _s_range(h, SM - 1)[1]  # = 4608


def build_nc(gelu_mode="gelu"):
    nc = bacc.Bacc("TRN2", target_bir_lowering=False, debug=False)

    def din(name, shape, dt=F32):
        return nc.dram_tensor(name, list(shape), dt, kind="ExternalInput").ap()

    d = {}
    d["xT"] = din("xT", (D, S), BF)
    d["w_in"] = din("w_in", (D, D), BF)
    d["b_in"] = din("b_in", (D,))
    d["wq"] = din("wq", (D, D), BF)
    d["wk"] = din("wk", (D, D), BF)
    d["wv"] = din("wv", (D, D), BF)
    d["wo"] = din("wo", (D, D), BF)
    d["bo"] = din("bo", (D,))
    d["w1"] = din("w1", (D, FFN), BF)
    d["w2"] = din("w2", (FFN, D), BF)
    d["b2"] = din("b2", (D,))
    d["w_out"] = din("w_out", (D, D), BF)
    d["b_out"] = din("b_out", (D,))
    d["bqc"] = din("bqc", (HD, H))
    d["b1c"] = din("b1c", (P, FK))
    d["bv"] = din("bv", (D,))
    d["qrow"] = din("qrow", (H, S), BF)
    d["tb"] = din("tb", (P, H * SM))
    d["maskf"] = din("maskf", (P, P), BF)
    d["maskb"] = din("maskb", (P, P), BF)
    d["ident"] = din("ident", (P, P), BF)
    d["out"] = nc.dram_tensor("out", [S, D], F32, kind="ExternalOutput").ap()

    with tile.TileContext(nc) as tc:
        _emit(nc, tc, d, gelu_mode)
    nc.compile()
    return nc


def _emit(nc, tc, d, gelu_mode):
    pool = tc.alloc_tile_pool

    pc = pool(name="consts", bufs=1)
    pw = pool(name="weights", bufs=1)  # all weights resident, bf16
    ph = pool(name="resid", bufs=2)  # tag "h": h1, h2, h3 rotate (fp32)
    phT = pool(name="transposed", bufs=2)  # tag "hT": hn1T,attnT2,hn2T,hn3T
    psm = pool(name="smalls", bufs=4)
    phn = pool(name="hn_nat", bufs=2)
    pg = pool(name="gelu", bufs=3)
    posb = pool(name="outsb", bufs=3)
    pattn = pool(name="attn_nat", bufs=1)
    pva = pool(name="vaug", bufs=1)
    pqk = pool(name="qkheads", bufs=2)
    pexp = pool(name="expT", bufs=2)

    ps_mm = pool(name="ps_mm", bufs=2, space="PSUM")
    # 4-deep rotation: score matmuls run ahead of ACT exp; doubles as the
    # 4 live FFN2 accumulators
    ps_acc = pool(name="ps_acc", bufs=4, space="PSUM")
    ps_tr = pool(name="ps_tr", bufs=2, space="PSUM")

    # ---- weights (bf16), staged early; all fit resident ----
    def wload(name, shape, view):
        t = pw.tile(shape, BF, tag=name)
        nc.sync.dma_start(out=t, in_=view)
        return t

    win_sb = wload("w_in", [P, DK, D], d["w_in"].rearrange("(c p) n -> p c n", p=P))
    # x arrives in s-chunks so h1(m=0) can start after 1/8 of the load
    xT_sb = pw.tile([P, DK, S], BF, tag="xT")
    xT_view = d["xT"].rearrange("(c p) n -> p c n", p=P)
    for m in range(SM):
        nc.sync.dma_start(
            out=xT_sb[:, :, m * P : (m + 1) * P],
            in_=xT_view[:, :, m * P : (m + 1) * P],
        )
    wq_sb = wload("wq", [P, DK, D], d["wq"].rearrange("(c p) n -> p c n", p=P))
    wk_sb = wload("wk", [P, DK, D], d["wk"].rearrange("(c p) n -> p c n", p=P))
    wv_sb = wload("wv", [P, DK, D], d["wv"].rearrange("(c p) n -> p c n", p=P))
    wo_sb = wload("wo", [P, DK, D], d["wo"].rearrange("(c p) n -> p c n", p=P))
    w1_sb = wload("w1", [P, DK, FFN], d["w1"].rearrange("(c p) n -> p c n", p=P))
    w2_sb = wload("w2", [P, FK, D], d["w2"].rearrange("(c p) n -> p c n", p=P))
    wout_sb = wload("w_out", [P, DK, D], d["w_out"].rearrange("(c p) n -> p c n", p=P))

    # ---- constants ----
    identB = pc.tile([P, P], BF, tag="ident")
    nc.sync.dma_start(out=identB, in_=d["ident"])
    maskf = pc.tile([P, P], BF, tag="maskf")
    nc.sync.dma_start(out=maskf, in_=d["maskf"])
    maskb = pc.tile([P, P], BF, tag="maskb")
    nc.sync.dma_start(out=maskb, in_=d["maskb"])
    tb = pc.tile([P, H * SM], F32, tag="tb")
    nc.sync.dma_start(out=tb, in_=d["tb"])
    bqc = pc.tile([HD, H], F32, tag="bqc")
    nc.sync.dma_start(out=bqc, in_=d["bqc"])
    b1c = pc.tile([P, FK], F32, tag="b1c")
    nc.sync.dma_start(out=b1c, in_=d["b1c"])
    b1cs = pc.tile([P, FK], F32, tag="b1cs")
    nc.any.tensor_scalar(b1cs, b1c, scalar1=1.702, scalar2=None, op0=OP.mult)

    def bcast(name, shape=None):
        t = pc.tile(shape or [P, D], F32, tag=name + "B")
        nc.gpsimd.dma_start(out=t, in_=d[name].partition_broadcast(P))
        return t

    epsc = pc.tile([P, 1], F32, tag="epsc")
    nc.any.memset(epsc, EPS)

    binB = bcast("b_in")
    bvB = bcast("bv", [P, H, HD])
    boB = bcast("bo")
    b2B = bcast("b2")
    boutB = bcast("b_out")

    # ---- h1 = x @ w_in + b_in  (natural fp32, residual base) ----
    h1 = ph.tile([P, SM, D], F32, tag="h")
    for m in range(SM):
        ps = ps_mm.tile([P, D], F32, tag="mm")
        for dk in range(DK):
            nc.tensor.matmul(
                ps,
                xT_sb[:, dk, m * P : (m + 1) * P],
                win_sb[:, dk, :],
                start=(dk == 0),
                stop=(dk == DK - 1),
            )
        nc.vector.tensor_tensor(out=h1[:, m, :], in0=ps, in1=binB, op=OP.add)

    def ln_chunk(src):
        # plain LayerNorm (no scale/bias; those are folded into weights)
        stats = psm.tile([P, 6], F32, tag="st")
        nc.vector.bn_stats(stats, src)
        mv = psm.tile([P, 2], F32, tag="mv")
        nc.vector.bn_aggr(mv, stats)
        sq = psm.tile([P, 1], F32, tag="sq")
        nc.scalar.activation(sq, mv[:, 1:2], AF.Sqrt, bias=epsc)
        rstd = psm.tile([P, 1], F32, tag="rstd")
        nc.vector.reciprocal(rstd, sq)
        negmr = psm.tile([P, 1], F32, tag="negmr")
        nc.vector.tensor_scalar(
            negmr, mv[:, 0:1], scalar1=rstd, scalar2=-1.0, op0=OP.mult, op1=OP.mult
        )
        hn = phn.tile([P, D], BF, tag="hn")
        nc.scalar.activation(hn, src, AF.Identity, bias=negmr, scale=rstd)
        return hn

    def transpose_row(hT, m, src):
        # transpose the 4 [128,128] blocks of src into one psum tile, then
        # write hT[:, :, m*P:(m+1)*P] with a single strided DVE copy
        t4 = ps_tr.tile([P, DK, P], BF, tag="tr")
        for dk in range(DK):
            nc.tensor.transpose(
                t4[:, dk, :], src[:, dk * P : (dk + 1) * P], identB
            )
        nc.vector.tensor_copy(hT[:, :, m * P : (m + 1) * P], t4)

    def make_hnT(hsrc):
        hT = phT.tile([P, DK, S], BF, tag="hT")
        for m in range(SM):
            hn = ln_chunk(hsrc[:, m, :])
            transpose_row(hT, m, hn)
        return hT

    # hn1T = LN1(h1) transposed [d, s] bf16
    hn1T = make_hnT(h1)

    # ---- v projection -> v_aug [P=t, SM, H, 65] bf16 (ones col for denom) ----
    v_aug = pva.tile([P, SM, H, HD + 1], BF, tag="vaug")
    for t in range(SM):
        psv = ps_mm.tile([P, H, HD], F32, tag="mm", name="psv")
        for dk in range(DK):
            nc.tensor.matmul(
                psv,
                hn1T[:, dk, t * P : (t + 1) * P],
                wv_sb[:, dk, :],
                start=(dk == 0),
                stop=(dk == DK - 1),
            )
        nc.vector.tensor_tensor(out=v_aug[:, t, :, 0:HD], in0=psv, in1=bvB, op=OP.add)
        nc.gpsimd.memset(v_aug[:, t, :, HD : HD + 1], 1.0)

    # ---- attention: software-pipelined so PE never waits on ACT exp ----
    # issue order per step: qk GEMMs(h), scores(h-1), probs@V(h-2); the exp
    # of head h-1 runs on ACT while PE does head h's projections.
    attn_nat = pattn.tile([P, SM, D], BF, tag="attn")
    qk_t = {}
    exp_t = {}

    def emit_qk(h):
        qTa = pqk.tile([HD + 1, S], BF, tag="qTa", name=f"qTa{h}")
        nc.sync.dma_start(out=qTa[HD : HD + 1, :], in_=d["qrow"][h : h + 1, :])
        kTa = pqk.tile([HD + 1, S], BF, tag="kTa", name=f"kTa{h}")
        nc.gpsimd.memset(kTa[HD : HD + 1, :], 1.0)
        for w_sb, dst, is_q in ((wq_sb, qTa, True), (wk_sb, kTa, False)):
            for half in range(2):
                psq = ps_mm.tile([HD, D], F32, tag="mm", name="psq")
                for dk in range(DK):
                    nc.tensor.matmul(
                        psq,
                        w_sb[:, dk, h * HD : (h + 1) * HD],
                        hn1T[:, dk, half * 512 : (half + 1) * 512],
                        start=(dk == 0),
                        stop=(dk == DK - 1),
                    )
                if is_q:
                    nc.vector.tensor_scalar(
                        dst[0:HD, half * 512 : (half + 1) * 512],
                        psq,
                        scalar1=bqc[:, h : h + 1],
                        scalar2=None,
                        op0=OP.add,
                    )
                else:
                    # k bias dropped: it only shifts scores by a per-s
                    # constant, which softmax normalization cancels exactly
                    nc.vector.tensor_copy(
                        dst[0:HD, half * 512 : (half + 1) * 512], psq
                    )
        qk_t[h] = (qTa, kTa)

    def emit_scores(h):
        qTa, kTa = qk_t[h]
        expT = pexp.tile([P, _ewidth(h)], BF, tag="expT", name=f"expT{h}")
        for j in range(SM):
            s0, w = _s_range(h, j)
            eo = _eoff(h, j)
            off = 0
            while off < w:
                pw_ = min(512, w - off)
                pss = ps_acc.tile([P, pw_], F32, tag="acc", name="pss")
                nc.tensor.matmul(
                    pss,
                    kTa[:, j * P : (j + 1) * P],
                    qTa[:, s0 + off : s0 + off + pw_],
                    start=True,
                    stop=True,
                )
                nc.scalar.activation(
                    expT[:, eo + off : eo + off + pw_],
                    pss,
                    AF.Exp,
                    bias=tb[:, h * SM + j : h * SM + j + 1],
                    scale=0.125,
                )
                off += pw_
            # mask the diagonal 128x128 block (keep t<=s fwd / t>=s bwd)
            dg = eo if _fwd(h) else eo + j * P
            msk = maskf if _fwd(h) else maskb
            nc.gpsimd.tensor_tensor(
                out=expT[:, dg : dg + P],
                in0=expT[:, dg : dg + P],
                in1=msk,
                op=OP.mult,
            )
        exp_t[h] = expT

    def emit_pv(h):
        # probs @ V, s-major: out[s, 65]; col 64 = softmax denominator
        expT = exp_t.pop(h)
        qk_t.pop(h)
        for m in range(SM):
            js = list(range(0, m + 1)) if _fwd(h) else list(range(m, SM))
            pv = ps_mm.tile([P, HD + 1], F32, tag="mm", name="pvps")
            for i, j in enumerate(js):
                s0, _w = _s_range(h, j)
                col = _eoff(h, j) + (m * P - s0)
                nc.tensor.matmul(
                    pv,
                    expT[:, col : col + P],
                    v_aug[:, j, h, :],
                    start=(i == 0),
                    stop=(i == len(js) - 1),
                )
            rinv = psm.tile([P, 1], F32, tag="rinv")
            nc.vector.reciprocal(rinv, pv[:, HD : HD + 1])
            nc.vector.tensor_scalar(
                attn_nat[:, m, h * HD : (h + 1) * HD],
                pv[:, 0:HD],
                scalar1=rinv,
                scalar2=None,
                op0=OP.mult,
            )

    for step in range(H + 2):
        if step < H:
            emit_qk(step)
        if 0 <= step - 1 < H:
            emit_scores(step - 1)
        if 0 <= step - 2 < H:
            emit_pv(step - 2)

    # ---- attnT2 + wo + h2 + LN2, per s-chunk; LN2 rows 4..7 deferred into
    # the FFN half-0 matmul stream so PE is never parked behind the LN chain
    attnT2 = phT.tile([P, DK, S], BF, tag="hT")
    h2 = ph.tile([P, SM, D], F32, tag="h")
    hn2T = phT.tile([P, DK, S], BF, tag="hT")

    def emit_wo_h2(m):
        transpose_row(attnT2, m, attn_nat[:, m, :])
        ps = ps_mm.tile([P, D], F32, tag="mm", name="pswo")
        for dk in range(DK):
            nc.tensor.matmul(
                ps,
                attnT2[:, dk, m * P : (m + 1) * P],
                wo_sb[:, dk, :],
                start=(dk == 0),
                stop=(dk == DK - 1),
            )
        nc.vector.tensor_tensor(out=h2[:, m, :], in0=ps, in1=h1[:, m, :], op=OP.add)
        nc.gpsimd.tensor_tensor(out=h2[:, m, :], in0=h2[:, m, :], in1=boB, op=OP.add)

    for m in range(SM):
        emit_wo_h2(m)
    for m in range(4):
        hn = ln_chunk(h2[:, m, :])
        transpose_row(hn2T, m, hn)

    # ---- FFN: h3 = h2 + gelu(hn2 @ w1 + b1) @ w2 + b2 ----
    h3 = ph.tile([P, SM, D], F32, tag="h")
    hn3T = phT.tile([P, DK, S], BF, tag="hT")
    out_view = d["out"].rearrange("(c p) n -> p c n", p=P)

    def emit_out_tail(m):
        # LNf row + output projection + store, interleaved into FFN half-1
        hn = ln_chunk(h3[:, m, :])
        transpose_row(hn3T, m, hn)
        ps = ps_mm.tile([P, D], F32, tag="mm", name="psout")
        for dk in range(DK):
            nc.tensor.matmul(
                ps,
                hn3T[:, dk, m * P : (m + 1) * P],
                wout_sb[:, dk, :],
                start=(dk == 0),
                stop=(dk == DK - 1),
            )
        osb = posb.tile([P, D], F32, tag="osb")
        nc.vector.tensor_tensor(out=osb, in0=ps, in1=boutB, op=OP.add)
        nc.sync.dma_start(out=out_view[:, m, :], in_=osb)

    def emit_h3(m, acc):
        nc.vector.tensor_tensor(out=h3[:, m, :], in0=acc, in1=h2[:, m, :], op=OP.add)
        nc.gpsimd.tensor_tensor(out=h3[:, m, :], in0=h3[:, m, :], in1=b2B, op=OP.add)

    for half in range(2):
        accs = []
        for mm in range(4):
            accs.append(ps_acc.tile([P, D], F32, tag="acc", name=f"ff2ps{mm}"))
        for kc in range(FK):
            ps1 = ps_mm.tile([P, 512], F32, tag="mm", name="ff1ps")
            for dk in range(DK):
                nc.tensor.matmul(
                    ps1,
                    w1_sb[:, dk, kc * P : (kc + 1) * P],
                    hn2T[:, dk, half * 512 : (half + 1) * 512],
                    start=(dk == 0),
                    stop=(dk == DK - 1),
                )
            gt = pg.tile([P, 512], BF, tag="gt")
            if gelu_mode == "gelu":
                nc.scalar.activation(gt, ps1, AF.Gelu, bias=b1c[:, kc : kc + 1])
            else:  # CoreSim lacks Gelu: x*sigmoid(1.702x) stand-in
                sg = pg.tile([P, 512], F32, tag="sg")
                nc.scalar.activation(
                    sg, ps1, AF.Sigmoid, bias=b1cs[:, kc : kc + 1], scale=1.702
                )
                xb = pg.tile([P, 512], F32, tag="xb")
                nc.any.tensor_scalar(
                    xb, ps1, scalar1=b1c[:, kc : kc + 1], scalar2=None, op0=OP.add
                )
                nc.any.tensor_tensor(out=gt, in0=sg, in1=xb, op=OP.mult)
            for mm in range(4):
                nc.tensor.matmul(
                    accs[mm],
                    gt[:, mm * P : (mm + 1) * P],
                    w2_sb[:, kc, :],
                    start=(kc == 0),
                    stop=(kc == FK - 1),
                )
            # interleave deferred LN rows / output tails into the MM stream
            if kc % 4 == 2:
                i = kc // 4
                if half == 0:
                    hn = ln_chunk(h2[:, 4 + i, :])
                    transpose_row(hn2T, 4 + i, hn)
                else:
                    emit_out_tail(i)
        for mm in range(4):
            emit_h3(half * 4 + mm, accs[mm])

    for m in range(4, SM):
        emit_out_tail(m)

    for p_ in (ps_tr, ps_acc, ps_mm, pexp, pqk, pva, pattn, posb, pg, phn,
               psm, phT, ph, pw, pc):
        p_.release()


def host_prep(inputs):
    """Fold LN affine params into weights; build ALiBi helper tensors."""
    f = lambda k: np.asarray(inputs[k], dtype=np.float64)
    ln1_s, ln1_b = f("ln1_s"), f("ln1_b")
    ln2_s, ln2_b = f("ln2_s"), f("ln2_b")
    lnf_s, lnf_b = f("lnf_s"), f("lnf_b")
    wq, bq = f("wq"), f("bq")
    wk = f("wk")
    wv, bv = f("wv"), f("bv")
    w1, b1 = f("w1"), f("b1")
    w_out, b_out = f("w_out"), f("b_out")

    wq_f = ln1_s[:, None] * wq
    bq_f = (bq + ln1_b @ wq).astype(np.float32)
    wk_f = ln1_s[:, None] * wk
    wv_f = ln1_s[:, None] * wv
    bv_f = (bv + ln1_b @ wv).astype(np.float32)
    w1_f = ln2_s[:, None] * w1
    b1_f = (b1 + ln2_b @ w1).astype(np.float32)
    wout_f = lnf_s[:, None] * w_out
    bout_f = (b_out + lnf_b @ w_out).astype(np.float32)

    sl = _slopes()
    qrow = np.zeros((H, S), np.float32)
    tb = np.zeros((P, H * SM), np.float32)
    s_idx = np.arange(S, dtype=np.float64)
    p_idx = np.arange(P, dtype=np.float64)
    for h in range(H):
        sgn = -1.0 if h < H // 2 else 1.0  # sign of the per-s row term
        qrow[h] = (sgn * 8.0 * sl[h % 4] * s_idx).astype(np.float32)
        for j in range(SM):
            tb[:, h * SM + j] = (-sgn * sl[h % 4] * (j * P + p_idx)).astype(
                np.float32
            )
    maskf = np.triu(np.ones((P, P), np.float32))  # keep t <= s (p <= c)
    maskb = np.tril(np.ones((P, P), np.float32))  # keep t >= s (p >= c)

    bf = lambda a: np.ascontiguousarray(np.asarray(a, np.float32).astype(BF_NP))
    common = {
        "w_in": bf(inputs["w_in"]),
        "b_in": np.asarray(inputs["b_in"], np.float32),
        "wq": bf(wq_f),
        "wk": bf(wk_f),
        "wv": bf(wv_f),
        "wo": bf(inputs["wo"]),
        "bo": np.asarray(inputs["bo"], np.float32),
        "w1": bf(w1_f),
        "w2": bf(inputs["w2"]),
        "b2": np.asarray(inputs["b2"], np.float32),
        "w_out": bf(wout_f),
        "b_out": bout_f,
        "bqc": np.ascontiguousarray(bq_f.reshape(H, HD).T),
        "b1c": np.ascontiguousarray(b1_f.reshape(FK, P).T),
        "bv": bv_f,
        "qrow": bf(qrow),
        "tb": tb,
        "maskf": bf(maskf),
        "maskb": bf(maskb),
        "ident": bf(np.eye(P, dtype=np.float32)),
    }
    return common


def core_map(common, x, i):
    xT = np.ascontiguousarray(np.asarray(x[i], np.float32).T.astype(BF_NP))
    return dict(common, xT=xT)


_NC_CACHE = {}


def get_nc(gelu_mode="gelu"):
    if gelu_mode not in _NC_CACHE:
        _NC_CACHE[gelu_mode] = build_nc(gelu_mode)
    return _NC_CACHE[gelu_mode]


def run(inputs, trace=False, tmpdir=None):
    common = host_prep(inputs)
    x = np.asarray(inputs["x"], np.float32)
    in_maps = [core_map(common, x, i) for i in range(N_CORES)]
    nc = get_nc()
    res = run_bass_kernel_spmd(
        nc, in_maps, core_ids=list(range(N_CORES)), trace=trace, tmpdir=tmpdir
    )
    out = np.stack([res.results[i]["out"] for i in range(N_CORES)])
    return out.astype(np.float32), res


def kernel(**inputs):
    out, _ = run(inputs, trace=False)
    return out
